# revision 6
# baseline (speedup 1.0000x reference)
"""ArcFace margin loss (ArcMarginLoss) on 8 Trainium2 NeuronCores.

Strategy (classification-parallel ArcFace):
  - Shard the class dimension V=32000 of W across 8 cores (4000 classes each,
    zero-padded to 4096 so every tile is 128 rows).
  - Each core: L2-normalize x (full batch) and its W shard in fp32, cast to
    bf16, transpose both to K-major via the PE, then compute its
    [2048 x 4096] block of cosine logits with bf16 matmuls accumulated in
    fp32 PSUM.  The exp(s*cos - 30) + row-sum is fused into one scalar-engine
    activation per PSUM chunk (accum_out).
  - Label-column correction: every core receives Wg = W[labels] (pure host
    gather) aligned 1:1 with x rows; computes cos_y = <nx_i, nWg_i> on
    device, phi_y = cos(theta + m), and folds
    (exp(s*phi-30) - exp(s*cos_y-30)) into its partial sum for the rows it
    owns (owned mask input).  It also outputs t_i = s*phi_i for owned rows.
  - Host epilogue: S = sum_c S_c, t = sum_c t_c,
    loss = mean(30 + log(S) - t).   (logits <= s = 30, so exp(l - 30) never
    overflows and the global max subtraction of a standard two-pass softmax
    is unnecessary -- no cross-core reduction needed on device.)

The zero-padded W rows produce exactly cos = 0 -> exp(-30) terms; the kernel
subtracts 96 * exp_act(-30) (computed with the same activation table) so the
padding cancels exactly.
"""

import numpy as np
from contextlib import ExitStack

import concourse.bass as bass
import concourse.tile as tile
from concourse import bacc, mybir
from concourse import bass_utils
from concourse._compat import with_exitstack
from concourse.masks import make_identity

P = 128
B = 2048          # batch rows
D = 512           # feature dim
V = 32000         # classes
NCORES = 8
VS = V // NCORES  # 4000 classes per core
VSP = 4096        # padded shard size (32 tiles of 128)
NPAD = VSP - VS   # 96 zero rows
MT = B // P       # 16 row tiles
KT = D // P       # 4 contraction tiles
WTILES = VSP // P  # 32 W tiles per core
NCHUNK = 512      # psum free dim per matmul group
NT = VSP // NCHUNK  # 8 chunks

S_SCALE = 30.0
M_MARGIN = 0.5
SHIFT = 30.0      # exp(logit - SHIFT): logits <= 30 so always <= 0
EPS = 1e-12

F32 = mybir.dt.float32
BF16 = mybir.dt.bfloat16
AX = mybir.AxisListType
OP = mybir.AluOpType
AF = mybir.ActivationFunctionType


@with_exitstack
def _arc_kernel(ctx: ExitStack, tc: tile.TileContext,
                x_d: bass.AP, w_d: bass.AP, wg_d: bass.AP, own_d: bass.AP,
                s_d: bass.AP, t_d: bass.AP):
    nc = tc.nc
    import math
    cos_m = math.cos(M_MARGIN)
    sin_m = math.sin(M_MARGIN)

    sb = ctx.enter_context(tc.tile_pool(name="sb", bufs=1))
    ld = ctx.enter_context(tc.tile_pool(name="ld", bufs=4))
    scr = ctx.enter_context(tc.tile_pool(name="scr", bufs=4))
    exs = ctx.enter_context(tc.tile_pool(name="exs", bufs=4))
    ps = ctx.enter_context(tc.tile_pool(name="ps", bufs=4, space="PSUM"))
    pst = ctx.enter_context(tc.tile_pool(name="pst", bufs=2, space="PSUM"))

    # persistent SBUF tensors
    nx = sb.tile([P, MT, D], BF16)       # normalized x, row-major
    nxT = sb.tile([P, KT, B], BF16)      # x^T (K-major)
    nwT = sb.tile([P, KT, VSP], BF16)    # W^T (K-major)
    ident = sb.tile([P, P], BF16)
    make_identity(nc, ident)

    nbias = sb.tile([P, 1], F32)         # -SHIFT bias for all the exp ops
    nc.vector.memset(nbias, -SHIFT)

    Spart = sb.tile([P, MT, NT], F32)    # per-chunk exp row sums
    Sacc = sb.tile([P, MT], F32)         # final partial sums per row
    cosy = sb.tile([P, MT], F32)         # cos at label column
    own_t = sb.tile([P, MT], F32)

    nc.sync.dma_start(out=own_t, in_=own_d.rearrange("(m p) -> p m", p=P))

    # ---- x chain: load, norms, normalize+cast, transpose ----
    def norm_chain(src_tile, col_out, use_act=True):
        """sumsq along rows -> 1/max(sqrt(ssq), eps) as [P,1] fp32."""
        sq = scr.tile([P, D], F32, tag="sq")
        ssq = scr.tile([P, 1], F32, tag="ssq")
        if use_act:
            nc.scalar.activation(out=sq, in_=src_tile, func=AF.Square,
                                 accum_out=ssq)
        else:
            nc.vector.scalar_tensor_tensor(
                out=sq, in0=src_tile, scalar=1.0, in1=src_tile,
                op0=OP.mult, op1=OP.mult, accum_out=ssq)
        rn = col_out
        nc.scalar.sqrt(rn, ssq)
        nc.vector.tensor_scalar_max(rn, rn, EPS)
        nc.vector.reciprocal(rn, rn)
        return rn

    rx = sb.tile([P, MT], F32)
    for m in range(MT):
        xt = ld.tile([P, D], F32, tag="xload")
        nc.sync.dma_start(out=xt, in_=x_d[m * P:(m + 1) * P, :])
        rn = norm_chain(xt, rx[:, m:m + 1])
        nc.gpsimd.tensor_scalar_mul(nx[:, m, :], xt, rn)
        for k in range(KT):
            pt = pst.tile([P, P], BF16, tag="tpsum")
            nc.tensor.transpose(pt, nx[:, m, k * P:(k + 1) * P], ident)
            if (m + k) % 2 == 0:
                nc.vector.tensor_copy(out=nxT[:, k, m * P:(m + 1) * P], in_=pt)
            else:
                nc.scalar.copy(nxT[:, k, m * P:(m + 1) * P], pt)

    # ---- W chain ----
    rw = sb.tile([P, WTILES], F32)
    for t in range(WTILES):
        wt = ld.tile([P, D], F32, tag="wload")
        nc.sync.dma_start(out=wt, in_=w_d[t * P:(t + 1) * P, :])
        rn = norm_chain(wt, rw[:, t:t + 1])
        nwr = scr.tile([P, D], BF16, tag="nwrow")
        nc.gpsimd.tensor_scalar_mul(nwr, wt, rn)
        for k in range(KT):
            pt = pst.tile([P, P], BF16, tag="tpsum")
            nc.tensor.transpose(pt, nwr[:, k * P:(k + 1) * P], ident)
            if (t + k) % 2 == 0:
                nc.vector.tensor_copy(out=nwT[:, k, t * P:(t + 1) * P], in_=pt)
            else:
                nc.scalar.copy(nwT[:, k, t * P:(t + 1) * P], pt)

    # ---- main loop: cosine matmul + fused exp/row-sum ----
    for m in range(MT):
        for n in range(NT):
            pm = ps.tile([P, NCHUNK], F32, tag="mm")
            for k in range(KT):
                nc.tensor.matmul(
                    pm,
                    nxT[:, k, m * P:(m + 1) * P],
                    nwT[:, k, n * NCHUNK:(n + 1) * NCHUNK],
                    start=(k == 0), stop=(k == KT - 1))
            ex = exs.tile([P, NCHUNK], F32, tag="ex")
            nc.scalar.activation(
                out=ex, in_=pm, func=AF.Exp,
                bias=nbias, scale=S_SCALE,
                accum_out=Spart[:, m, n:n + 1])
        nc.vector.tensor_reduce(
            out=Sacc[:, m:m + 1], in_=Spart[:, m, :], axis=AX.X, op=OP.add)

    # ---- label-column correction ----
    rg = sb.tile([P, MT], F32)
    for m in range(MT):
        gt = ld.tile([P, D], F32, tag="gload")
        nc.sync.dma_start(out=gt, in_=wg_d[m * P:(m + 1) * P, :])
        rn = norm_chain(gt, rg[:, m:m + 1], use_act=False)
        ngr = scr.tile([P, D], BF16, tag="ngrow")
        nc.gpsimd.tensor_scalar_mul(ngr, gt, rn)
        sq2 = scr.tile([P, D], F32, tag="sq2")
        nc.vector.scalar_tensor_tensor(
            out=sq2, in0=nx[:, m, :], scalar=1.0, in1=ngr,
            op0=OP.mult, op1=OP.mult, accum_out=cosy[:, m:m + 1])

    # e0 = exp_act(-30) computed with the same table as the main loop, so the
    # 96 zero-pad columns (cos exactly 0) cancel exactly.
    zt = sb.tile([P, 1], F32)
    nc.vector.memset(zt, 0.0)
    e0 = sb.tile([P, 1], F32)
    nc.scalar.activation(out=e0, in_=zt, func=AF.Exp, bias=nbias, scale=S_SCALE)
    nc.vector.tensor_scalar_mul(e0, e0, float(NPAD))
    nc.vector.tensor_scalar(Sacc, Sacc, e0, None, OP.subtract)

    # phi = cosy*cos_m - sin*sin_m ; computed as mphi = sin*sin_m - cosy*cos_m
    sq = sb.tile([P, MT], F32)
    nc.vector.tensor_tensor(sq, cosy, cosy, OP.mult)
    om = sb.tile([P, MT], F32)
    nc.vector.tensor_scalar(om, sq, -1.0, 1.0, OP.mult, OP.add)
    nc.vector.tensor_scalar_max(om, om, 0.0)
    sin = sb.tile([P, MT], F32)
    nc.scalar.sqrt(sin, om)
    cm = sb.tile([P, MT], F32)
    nc.vector.tensor_scalar_mul(cm, cosy, cos_m)
    mphi = sb.tile([P, MT], F32)
    nc.vector.scalar_tensor_tensor(
        out=mphi, in0=sin, scalar=sin_m, in1=cm, op0=OP.mult, op1=OP.subtract)

    expphi = sb.tile([P, MT], F32)
    nc.scalar.activation(out=expphi, in_=mphi, func=AF.Exp,
                         bias=nbias, scale=-S_SCALE)
    expcos = sb.tile([P, MT], F32)
    nc.scalar.activation(out=expcos, in_=cosy, func=AF.Exp,
                         bias=nbias, scale=S_SCALE)
    delta = sb.tile([P, MT], F32)
    nc.vector.tensor_tensor(delta, expphi, expcos, OP.subtract)
    nc.vector.tensor_tensor(delta, delta, own_t, OP.mult)
    nc.vector.tensor_tensor(Sacc, Sacc, delta, OP.add)

    tvec = sb.tile([P, MT], F32)
    nc.vector.tensor_scalar_mul(tvec, mphi, -S_SCALE)
    nc.vector.tensor_tensor(tvec, tvec, own_t, OP.mult)

    nc.sync.dma_start(out=s_d.rearrange("(m p) -> p m", p=P), in_=Sacc)
    nc.sync.dma_start(out=t_d.rearrange("(m p) -> p m", p=P), in_=tvec)


def build_bass():
    nc = bacc.Bacc("TRN2", target_bir_lowering=False, debug=False,
                   enable_asserts=False, num_devices=NCORES)
    x_d = nc.dram_tensor("x_in", [B, D], F32, kind="ExternalInput").ap()
    w_d = nc.dram_tensor("w_shard", [VSP, D], F32, kind="ExternalInput").ap()
    wg_d = nc.dram_tensor("w_gather", [B, D], F32, kind="ExternalInput").ap()
    own_d = nc.dram_tensor("owned", [B], F32, kind="ExternalInput").ap()
    s_d = nc.dram_tensor("s_out", [B], F32, kind="ExternalOutput").ap()
    t_d = nc.dram_tensor("t_out", [B], F32, kind="ExternalOutput").ap()
    with tile.TileContext(nc) as tc:
        _arc_kernel(tc, x_d, w_d, wg_d, own_d, s_d, t_d)
    nc.compile()
    return nc


_NC = None


def _get_nc():
    global _NC
    if _NC is None:
        _NC = build_bass()
    return _NC


def make_in_maps(x: np.ndarray, W: np.ndarray, labels: np.ndarray):
    x = np.ascontiguousarray(x, dtype=np.float32)
    W = np.ascontiguousarray(W, dtype=np.float32)
    lab = np.asarray(labels).astype(np.int64)
    wg = np.ascontiguousarray(W[lab])            # [B, D] host gather
    shard_of = lab // VS
    in_maps = []
    for c in range(NCORES):
        wsh = np.zeros((VSP, D), dtype=np.float32)
        wsh[:VS] = W[c * VS:(c + 1) * VS]
        owned = (shard_of == c).astype(np.float32)
        in_maps.append({
            "x_in": x,
            "w_shard": wsh,
            "w_gather": wg,
            "owned": owned,
        })
    return in_maps


def combine_outputs(results):
    S = np.zeros(B, dtype=np.float64)
    t = np.zeros(B, dtype=np.float64)
    for r in results:
        S += r["s_out"].astype(np.float64)
        t += r["t_out"].astype(np.float64)
    loss = np.mean(SHIFT + np.log(S) - t)
    return np.float32(loss)


def kernel(x, W, labels, **run_kwargs):
    nc = _get_nc()
    in_maps = make_in_maps(x, W, labels)
    res = bass_utils.run_bass_kernel_spmd(
        nc, in_maps, core_ids=list(range(NCORES)), **run_kwargs)
    out = combine_outputs(res.results)
    kernel.last_results = res
    return out


# revision 8
# speedup vs baseline: 2.8682x; 2.8682x over previous
"""ArcFace margin loss (ArcMarginLoss) on 8 Trainium2 NeuronCores.

Strategy (classification-parallel ArcFace):
  - Shard the class dimension V=32000 of W across 8 cores (4000 classes each,
    zero-padded to 4096 so every tile is 128 rows).
  - Each core: L2-normalize x (full batch) and its W shard in fp32, cast to
    bf16, transpose both to K-major via the PE, then compute its
    [2048 x 4096] block of cosine logits with bf16 matmuls accumulated in
    fp32 PSUM.  The exp(s*cos - 30) + row-sum is fused into one scalar-engine
    activation per 1024-wide PSUM chunk (accum_out).
  - Label-column correction: every core receives Wg = W[labels] (pure host
    gather) aligned 1:1 with x rows; computes cos_y = <nx_i, nWg_i> on
    device, phi_y = cos(theta + m), and folds
    (exp(s*phi-30) - exp(s*cos_y-30)) into its partial sum for the rows it
    owns (owned mask input).  It also outputs t_i = s*phi_i for owned rows.
  - Host epilogue: S = sum_c S_c, t = sum_c t_c,
    loss = mean(30 + log(S) - t).   (logits <= s = 30, so exp(l - 30) never
    overflows and the global max subtraction of a standard two-pass softmax
    is unnecessary -- no cross-core reduction needed on device.)

The zero-padded W rows produce exactly cos = 0 -> exp(-30) terms; the kernel
subtracts 96 * exp_act(-30) (computed with the same activation table) so the
padding cancels exactly.
"""

import math
import numpy as np
from contextlib import ExitStack

import concourse.bass as bass
import concourse.tile as tile
from concourse import bacc, mybir
from concourse import bass_utils
from concourse._compat import with_exitstack
from concourse.masks import make_identity

P = 128
B = 2048          # batch rows
D = 512           # feature dim
V = 32000         # classes
NCORES = 8
VS = V // NCORES  # 4000 classes per core
VSP = 4096        # padded shard size (32 tiles of 128)
NPAD = VSP - VS   # 96 zero rows
MT = B // P       # 16 row tiles
KT = D // P       # 4 contraction tiles
WTILES = VSP // P  # 32 W tiles per core
NCHUNK = 1024     # exp chunk width (2 PSUM banks)
NT = VSP // NCHUNK  # 4 chunks
GB = 8            # tiles per batched-norm group

S_SCALE = 30.0
M_MARGIN = 0.5
SHIFT = 30.0      # exp(logit - SHIFT): logits <= 30 so always <= 0
EPS = 1e-12

F32 = mybir.dt.float32
BF16 = mybir.dt.bfloat16
AX = mybir.AxisListType
OP = mybir.AluOpType
AF = mybir.ActivationFunctionType


@with_exitstack
def _arc_kernel(ctx: ExitStack, tc: tile.TileContext,
                x_d: bass.AP, w_d: bass.AP, wg_d: bass.AP, own_d: bass.AP,
                s_d: bass.AP, t_d: bass.AP):
    nc = tc.nc
    cos_m = math.cos(M_MARGIN)
    sin_m = math.sin(M_MARGIN)

    sb = ctx.enter_context(tc.tile_pool(name="sb", bufs=1))
    ld = ctx.enter_context(tc.tile_pool(name="ld", bufs=10))
    scr = ctx.enter_context(tc.tile_pool(name="scr", bufs=4))
    exs = ctx.enter_context(tc.tile_pool(name="exs", bufs=3))
    ps = ctx.enter_context(tc.tile_pool(name="ps", bufs=3, space="PSUM"))
    pst = ctx.enter_context(tc.tile_pool(name="pst", bufs=2, space="PSUM"))

    # persistent SBUF tensors
    nx = sb.tile([P, MT, D], BF16)       # normalized x, row-major
    nxT = sb.tile([P, KT, B], BF16)      # x^T (K-major)
    nwT = sb.tile([P, KT, VSP], BF16)    # W^T (K-major)
    ident = sb.tile([P, P], BF16)
    make_identity(nc, ident)

    nbias = sb.tile([P, 1], F32)         # -SHIFT bias for all the exp ops
    nc.vector.memset(nbias, -SHIFT)

    Spart = sb.tile([P, MT, NT], F32)    # per-chunk exp row sums
    Sacc = sb.tile([P, MT], F32)         # final partial sums per row
    cosy = sb.tile([P, MT], F32)         # cos at label column
    own_t = sb.tile([P, MT], F32)

    nc.sync.dma_start(out=own_t, in_=own_d.rearrange("(m p) -> p m", p=P))

    def sumsq(src_tile, ssq_col):
        """row sum-of-squares via one DVE op (scratch out is discarded)."""
        sq = scr.tile([P, D], F32, tag="sq")
        nc.vector.scalar_tensor_tensor(
            out=sq, in0=src_tile, scalar=1.0, in1=src_tile,
            op0=OP.mult, op1=OP.mult, accum_out=ssq_col)

    def finish_norms(ssq_group, rn_group):
        """rn = 1/max(sqrt(ssq), eps), batched over a [P, gb] group."""
        nc.scalar.sqrt(rn_group, ssq_group)
        nc.vector.tensor_scalar_max(rn_group, rn_group, EPS)
        nc.vector.reciprocal(rn_group, rn_group)

    def transpose_tile(row_tile, dstT, col0):
        """PE-transpose a [P, D] bf16 tile into dstT[:, :, col0:col0+P]."""
        pt = pst.tile([P, KT, P], BF16, tag="tpsum")
        for k in range(KT):
            nc.tensor.transpose(pt[:, k], row_tile[:, k * P:(k + 1) * P], ident)
        nc.vector.tensor_copy(out=dstT[:, :, col0:col0 + P], in_=pt)

    # ---- x chain ----
    rx = sb.tile([P, MT], F32)
    xrows = []
    for m in range(MT):
        xt = ld.tile([P, D], F32, tag="xload")
        nc.sync.dma_start(out=xt, in_=x_d[m * P:(m + 1) * P, :])
        xrows.append(xt)
        sumsq(xt, rx[:, m:m + 1])
        if m % GB == GB - 1:
            g0 = m - (GB - 1)
            finish_norms(rx[:, g0:m + 1], rx[:, g0:m + 1])
            for mm in range(g0, m + 1):
                nc.vector.tensor_scalar_mul(nx[:, mm, :], xrows[mm],
                                            rx[:, mm:mm + 1])
                transpose_tile(nx[:, mm, :], nxT, mm * P)
                xrows[mm] = None

    # ---- W chain ----
    rw = sb.tile([P, WTILES], F32)
    wrows = []
    for t in range(WTILES):
        wt = ld.tile([P, D], F32, tag="wload")
        nc.sync.dma_start(out=wt, in_=w_d[t * P:(t + 1) * P, :])
        wrows.append(wt)
        sumsq(wt, rw[:, t:t + 1])
        if t % GB == GB - 1:
            g0 = t - (GB - 1)
            finish_norms(rw[:, g0:t + 1], rw[:, g0:t + 1])
            for tt in range(g0, t + 1):
                nwr = scr.tile([P, D], BF16, tag="nwrow")
                nc.vector.tensor_scalar_mul(nwr, wrows[tt], rw[:, tt:tt + 1])
                transpose_tile(nwr, nwT, tt * P)
                wrows[tt] = None

    # ---- label-gather chain (cos_y) ----
    rg = sb.tile([P, MT], F32)
    grows = []
    for m in range(MT):
        gt = ld.tile([P, D], F32, tag="gload")
        nc.sync.dma_start(out=gt, in_=wg_d[m * P:(m + 1) * P, :])
        grows.append(gt)
        sumsq(gt, rg[:, m:m + 1])
        if m % GB == GB - 1:
            g0 = m - (GB - 1)
            finish_norms(rg[:, g0:m + 1], rg[:, g0:m + 1])
            for mm in range(g0, m + 1):
                ngr = scr.tile([P, D], BF16, tag="ngrow")
                nc.vector.tensor_scalar_mul(ngr, grows[mm], rg[:, mm:mm + 1])
                sq2 = scr.tile([P, D], F32, tag="sq2")
                nc.vector.scalar_tensor_tensor(
                    out=sq2, in0=nx[:, mm, :], scalar=1.0, in1=ngr,
                    op0=OP.mult, op1=OP.mult, accum_out=cosy[:, mm:mm + 1])
                grows[mm] = None

    # ---- main loop: cosine matmul + fused exp/row-sum ----
    HALF = 512
    for m in range(MT):
        for n in range(NT):
            pm = ps.tile([P, NCHUNK], F32, tag="mm")
            for h in range(2):
                for k in range(KT):
                    nc.tensor.matmul(
                        pm[:, h * HALF:(h + 1) * HALF],
                        nxT[:, k, m * P:(m + 1) * P],
                        nwT[:, k, (n * 2 + h) * HALF:(n * 2 + h + 1) * HALF],
                        start=(k == 0), stop=(k == KT - 1))
            ex = exs.tile([P, NCHUNK], F32, tag="ex")
            nc.scalar.activation(
                out=ex, in_=pm, func=AF.Exp,
                bias=nbias, scale=S_SCALE,
                accum_out=Spart[:, m, n:n + 1])
        nc.vector.tensor_reduce(
            out=Sacc[:, m:m + 1], in_=Spart[:, m, :], axis=AX.X, op=OP.add)

    # e0 = exp_act(-30) computed with the same table as the main loop, so the
    # 96 zero-pad columns (cos exactly 0) cancel exactly.
    zt = sb.tile([P, 1], F32)
    nc.vector.memset(zt, 0.0)
    e0 = sb.tile([P, 1], F32)
    nc.scalar.activation(out=e0, in_=zt, func=AF.Exp, bias=nbias, scale=S_SCALE)
    nc.vector.tensor_scalar_mul(e0, e0, float(NPAD))
    nc.vector.tensor_scalar(Sacc, Sacc, e0, None, OP.subtract)

    # phi = cosy*cos_m - sin*sin_m ; computed as mphi = sin*sin_m - cosy*cos_m
    sq = sb.tile([P, MT], F32)
    nc.vector.tensor_tensor(sq, cosy, cosy, OP.mult)
    om = sb.tile([P, MT], F32)
    nc.vector.tensor_scalar(om, sq, -1.0, 1.0, OP.mult, OP.add)
    nc.vector.tensor_scalar_max(om, om, 0.0)
    sin = sb.tile([P, MT], F32)
    nc.scalar.sqrt(sin, om)
    cm = sb.tile([P, MT], F32)
    nc.vector.tensor_scalar_mul(cm, cosy, cos_m)
    mphi = sb.tile([P, MT], F32)
    nc.vector.scalar_tensor_tensor(
        out=mphi, in0=sin, scalar=sin_m, in1=cm, op0=OP.mult, op1=OP.subtract)

    expphi = sb.tile([P, MT], F32)
    nc.scalar.activation(out=expphi, in_=mphi, func=AF.Exp,
                         bias=nbias, scale=-S_SCALE)
    expcos = sb.tile([P, MT], F32)
    nc.scalar.activation(out=expcos, in_=cosy, func=AF.Exp,
                         bias=nbias, scale=S_SCALE)
    delta = sb.tile([P, MT], F32)
    nc.vector.tensor_tensor(delta, expphi, expcos, OP.subtract)
    nc.vector.tensor_tensor(delta, delta, own_t, OP.mult)
    nc.vector.tensor_tensor(Sacc, Sacc, delta, OP.add)

    tvec = sb.tile([P, MT], F32)
    nc.vector.tensor_scalar_mul(tvec, mphi, -S_SCALE)
    nc.vector.tensor_tensor(tvec, tvec, own_t, OP.mult)

    nc.sync.dma_start(out=s_d.rearrange("(m p) -> p m", p=P), in_=Sacc)
    nc.sync.dma_start(out=t_d.rearrange("(m p) -> p m", p=P), in_=tvec)


def build_bass():
    nc = bacc.Bacc("TRN2", target_bir_lowering=False, debug=False,
                   enable_asserts=False, num_devices=NCORES)
    x_d = nc.dram_tensor("x_in", [B, D], F32, kind="ExternalInput").ap()
    w_d = nc.dram_tensor("w_shard", [VSP, D], F32, kind="ExternalInput").ap()
    wg_d = nc.dram_tensor("w_gather", [B, D], F32, kind="ExternalInput").ap()
    own_d = nc.dram_tensor("owned", [B], F32, kind="ExternalInput").ap()
    s_d = nc.dram_tensor("s_out", [B], F32, kind="ExternalOutput").ap()
    t_d = nc.dram_tensor("t_out", [B], F32, kind="ExternalOutput").ap()
    with tile.TileContext(nc) as tc:
        _arc_kernel(tc, x_d, w_d, wg_d, own_d, s_d, t_d)
    nc.compile()
    return nc


_NC = None


def _get_nc():
    global _NC
    if _NC is None:
        _NC = build_bass()
    return _NC


def make_in_maps(x: np.ndarray, W: np.ndarray, labels: np.ndarray):
    x = np.ascontiguousarray(x, dtype=np.float32)
    W = np.ascontiguousarray(W, dtype=np.float32)
    lab = np.asarray(labels).astype(np.int64)
    wg = np.ascontiguousarray(W[lab])            # [B, D] host gather
    shard_of = lab // VS
    in_maps = []
    for c in range(NCORES):
        wsh = np.zeros((VSP, D), dtype=np.float32)
        wsh[:VS] = W[c * VS:(c + 1) * VS]
        owned = (shard_of == c).astype(np.float32)
        in_maps.append({
            "x_in": x,
            "w_shard": wsh,
            "w_gather": wg,
            "owned": owned,
        })
    return in_maps


def combine_outputs(results):
    S = np.zeros(B, dtype=np.float64)
    t = np.zeros(B, dtype=np.float64)
    for r in results:
        S += r["s_out"].astype(np.float64)
        t += r["t_out"].astype(np.float64)
    loss = np.mean(SHIFT + np.log(S) - t)
    return np.float32(loss)


def kernel(x, W, labels, **run_kwargs):
    nc = _get_nc()
    in_maps = make_in_maps(x, W, labels)
    res = bass_utils.run_bass_kernel_spmd(
        nc, in_maps, core_ids=list(range(NCORES)), **run_kwargs)
    out = combine_outputs(res.results)
    kernel.last_results = res
    return out
